# revision 1
# baseline (speedup 1.0000x reference)
"""Trainium2 Bass kernel for nn_DenseAttnProcessor (sparse_attention).

Cross-attention block: q = hs@Wq, k/v = ehs@{Wk,Wv}, per-head softmax((q k^T)/8
+ col_bias) @ v, @Wo + bo + residual.  B=8 batches -> data-parallel, one batch
per NeuronCore (no collectives).

Per-core dataflow (everything lives in "transposed" orientation so that every
matmul contraction has its operand already partition-major; softmax runs on
scoresT [T, q] with the per-head denominator handled by a ones-column matmul, a
reciprocal, and a K=1 broadcast matmul):

  stage A (once):  ehsT (host-pretransposed, bf16) -> k,v [77,1024] via matmul
                   -> kT via PE transpose -> M_h = v_h @ Wo_h [77,1024];
                   M rows DMA-packed into a [16*77+1, 1024] stack; the +bo
                   term rides as an extra stack row paired with an all-ones
                   probs row.
  stage B (8 chunks of 512 q rows):
                   hs chunk f32 -> bf16 cast -> XBAR DMA-transpose -> hsT [C, q]
                   qT = Wq^T@hsT (psum accum over C) [inner, q]
                   per head: scoresT [77,512] = kT_h^T qT_h; z = Exp(scoresT)
                   * exp(col_bias)^T (host-precomputed multiplicative mask,
                   exact "set-column" suppression semantics, rows without
                   suppression are exactly 1.0); D = ones^T z; Dinv via fast
                   DVE reciprocal; DinvB via K=1 broadcast matmul; probsT =
                   z * DinvB, DMA-packed into the [16*77+1, 512] stack;
                   out[q,C] = sum_kt probsT_kt^T @ M_kt (10 K=128 matmuls)
                   + residual (f32) -> DMA out.  Chunks are software-
                   pipelined: softmax(ci-1) is emitted interleaved with
                   qT(ci) so the PE stream stays dense (HAM stays warm).

Inputs are the full unsharded arrays as produced by setup_inputs(); host side
only shards/casts/transposes small tensors and computes the tiny [2,77]/[2,4096]
suppression vectors.
"""

import sys

for _p in ("/opt/trn_rl_repo",):
    if _p not in sys.path:
        sys.path.insert(0, _p)

import numpy as np
import ml_dtypes

import concourse.mybir as mybir
import concourse.tile as tile
from concourse import bacc
from concourse.bass import ds
from concourse.masks import make_identity

F32 = mybir.dt.float32
BF16 = mybir.dt.bfloat16
AF = mybir.ActivationFunctionType

B, HW, C, CT, T, H, D = 8, 4096, 1024, 2048, 77, 16, 64
SUPPRESS = 20.0
RT = H * T + 1                # 1233 stacked rows (16*77 head rows + bo row)
NKT = (RT + 127) // 128       # 10 K-tiles for the AV matmul
NQ = 512                      # q rows per chunk
NCHUNK = HW // NQ             # 8
BO_TILE, BO_PART = (H * T) // 128, (H * T) % 128   # bo/ones row: tile 9, p 80


def _pack_pieces(h):
    """DMA pieces for packing head h's 77 rows at stacked row 77*h, split at
    128-row tile boundaries.  Returns list of (tile_idx, part_base, src_start,
    nrows).  (DMA writes have no partition-alignment restrictions.)"""
    g = T * h
    pieces = []
    pos = 0
    while pos < T:
        gg = g + pos
        ti, d = gg // 128, gg % 128
        n = min(T - pos, 128 - d)
        pieces.append((ti, d, pos, n))
        pos += n
    return pieces


def build_nc():
    nc = bacc.Bacc("TRN2", target_bir_lowering=False, debug=False)

    hs = nc.dram_tensor("hs", [HW, C], F32, kind="ExternalInput")
    ehsT = nc.dram_tensor("ehsT", [CT, T], BF16, kind="ExternalInput")
    wq = nc.dram_tensor("wq", [C, C], BF16, kind="ExternalInput")
    wk = nc.dram_tensor("wk", [CT, C], BF16, kind="ExternalInput")
    wv = nc.dram_tensor("wv", [CT, C], BF16, kind="ExternalInput")
    wo = nc.dram_tensor("wo", [C, C], BF16, kind="ExternalInput")
    euabt = nc.dram_tensor("euabt", [T, HW], BF16, kind="ExternalInput")
    bo = nc.dram_tensor("bo", [1, C], BF16, kind="ExternalInput")
    out = nc.dram_tensor("out", [HW, C], F32, kind="ExternalOutput")

    with tile.TileContext(nc) as tc:
        with (
            tc.tile_pool(name="const", bufs=1) as const,
            tc.tile_pool(name="persist", bufs=1) as persist,
        ):
            ident = const.tile([128, 128], BF16)
            make_identity(nc, ident)
            ones_col = const.tile([T, 1], BF16)
            nc.any.memset(ones_col, 1.0)
            ones_row = const.tile([1, T], BF16)
            nc.any.memset(ones_row, 1.0)
            ones_q = const.tile([1, NQ], BF16)
            nc.any.memset(ones_q, 1.0)
            eu_sb = const.tile([T, HW], BF16)
            nc.sync.dma_start(eu_sb, euabt[:, :])

            # persistent stacks
            kT_sb = persist.tile([128, C // 128, T], BF16)        # [inner, t]
            m_tiles = [persist.tile([128, C], BF16, name=f"m{i}") for i in range(NKT)]
            prob_bufs = [
                [persist.tile([128, NQ], BF16, name=f"pb{b}_{i}") for i in range(NKT)]
                for b in range(2)
            ]
            wq_tiles = [persist.tile([128, C], BF16, name=f"wqt{i}") for i in range(C // 128)]
            for i in range(C // 128):
                nc.sync.dma_start(wq_tiles[i], wq[ds(128 * i, 128), :])

            # only the last stack tile has rows past the packed head rows;
            # zero it so the AV matmuls see zeros there, then land bo/ones.
            nc.any.memset(m_tiles[BO_TILE], 0.0)
            for bset in prob_bufs:
                nc.any.memset(bset[BO_TILE], 0.0)
                nc.sync.dma_start(
                    bset[BO_TILE][BO_PART : BO_PART + 1, :], ones_q
                )
            nc.sync.dma_start(m_tiles[BO_TILE][BO_PART : BO_PART + 1, :], bo[:, :])

            # ---------------- stage A: k, v, kT, M ----------------
            with (
                tc.tile_pool(name="sa_sb", bufs=3) as sa_sb,
                tc.tile_pool(name="sa_w", bufs=3) as sa_w,
                tc.tile_pool(name="sa_ps", bufs=2, space="PSUM") as sa_ps,
            ):
                ehsT_sb = sa_sb.tile([128, CT // 128, T], BF16, bufs=1)
                for j in range(CT // 128):
                    nc.sync.dma_start(ehsT_sb[:, j, :], ehsT[ds(128 * j, 128), :])

                kv_sb = {}
                for name, wten in (("k", wk), ("v", wv)):
                    kv_ps = sa_ps.tile([T, C], F32, tag="kvps", bufs=1)
                    for j in range(CT // 128):
                        wt = sa_w.tile([128, C], BF16, tag="wkv")
                        nc.sync.dma_start(wt, wten[ds(128 * j, 128), :])
                        for nh in range(2):
                            nc.tensor.matmul(
                                kv_ps[:, ds(512 * nh, 512)],
                                ehsT_sb[:, j, :],
                                wt[:, ds(512 * nh, 512)],
                                start=(j == 0),
                                stop=(j == CT // 128 - 1),
                            )
                    kvs = sa_sb.tile([T, C], BF16, tag=f"{name}sb", bufs=1)
                    nc.any.tensor_copy(kvs, kv_ps)
                    kv_sb[name] = kvs

                # kT / vT via PE transpose of 128-column slices
                vT_sb = sa_sb.tile([128, C // 128, T], BF16, bufs=1)
                for src, dst in ((kv_sb["k"], kT_sb), (kv_sb["v"], vT_sb)):
                    for i in range(C // 128):
                        tp = sa_ps.tile([128, T], BF16, tag="tpa")
                        nc.tensor.transpose(tp, src[:, ds(128 * i, 128)], ident[:T, :T])
                        nc.any.tensor_copy(dst[:, i, :], tp)

                # M_h = v_h @ Wo_h, packed at stacked row 96h (+ bo at row 95)
                wot = None
                for h in range(H):
                    i, po = h // 2, (h % 2) * 64
                    if h % 2 == 0:
                        wot = sa_w.tile([128, C], BF16, tag="wot")
                        nc.sync.dma_start(wot, wo[ds(128 * i, 128), :])
                    m_ps = sa_ps.tile([T, C], F32, tag="mps")
                    for nh in range(2):
                        nc.tensor.matmul(
                            m_ps[:, ds(512 * nh, 512)],
                            vT_sb[ds(po, 64), i, :],
                            wot[ds(po, 64), ds(512 * nh, 512)],
                            start=True,
                            stop=True,
                        )
                    m_stg = sa_sb.tile([T, C], BF16, tag="mstg")
                    nc.any.tensor_copy(m_stg, m_ps)
                    for (ti, pb, s0, nr) in _pack_pieces(h):
                        nc.gpsimd.dma_start(
                            m_tiles[ti][ds(pb, nr), :], m_stg[ds(s0, nr), :]
                        )

            # ---------------- stage B: software-pipelined q chunks ----------------
            # Engine streams execute in emission order, so softmax(ci-1) head
            # chains are interleaved with qT(ci) matmul groups at build time:
            # the PE stream then always has dense matmul work queued and the
            # HAM clock gate stays open.
            with (
                tc.tile_pool(name="hsp", bufs=2) as hsp,
                tc.tile_pool(name="work", bufs=2) as work,
                tc.tile_pool(name="soft", bufs=4) as soft,
                tc.tile_pool(name="ops", bufs=2, space="PSUM") as ops,
            ):
                st = {}

                def load(ci):
                    q0 = NQ * ci
                    hs_f = hsp.tile([128, NQ // 128, C], F32, tag="hsf")
                    for qj in range(NQ // 128):
                        nc.sync.dma_start(
                            hs_f[:, qj, :], hs[ds(q0 + 128 * qj, 128), :]
                        )
                    hs_bf = work.tile([128, NQ // 128, C], BF16, tag="hsbf")
                    for qj in range(NQ // 128):
                        nc.scalar.copy(hs_bf[:, qj, :], hs_f[:, qj, :])
                    hsT = work.tile([128, C // 128, NQ], BF16, tag="hsT")
                    for qj in range(NQ // 128):
                        nc.sync.dma_start(
                            hsT[:, :, ds(128 * qj, 128)],
                            hs_bf[:, qj, :],
                            transpose=True,
                        )
                    qT = work.tile([128, C // 128, NQ], BF16, tag="qT")
                    st[ci] = dict(hs_f=hs_f, hsT=hsT, qT=qT)

                def qt_group(ci, ij):
                    hsT, qT = st[ci]["hsT"], st[ci]["qT"]
                    q_ps = ops.tile([128, NQ], F32, tag="qps", bufs=1)
                    for cj in range(C // 128):
                        nc.tensor.matmul(
                            q_ps,
                            wq_tiles[cj][:, ds(128 * ij, 128)],
                            hsT[:, cj, :],
                            start=(cj == 0),
                            stop=(cj == C // 128 - 1),
                        )
                    nc.any.tensor_copy(qT[:, ij, :], q_ps)

                def sm_head1(ci, h):
                    q0 = NQ * ci
                    qT = st[ci]["qT"]
                    i, po = h // 2, (h % 2) * 64
                    sT_ps = ops.tile([T, NQ], F32, tag="sT", bufs=2)
                    nc.tensor.matmul(
                        sT_ps,
                        kT_sb[ds(po, 64), i, :],
                        qT[ds(po, 64), i, :],
                        start=True,
                        stop=True,
                    )
                    expT = soft.tile([T, NQ], BF16, tag="expT", bufs=4)
                    nc.scalar.activation(expT, sT_ps, AF.Exp)
                    # multiplicative suppression mask exp(col_bias^T), host-
                    # precomputed; rows without suppression are exactly 1.0
                    z = soft.tile([T, NQ], BF16, tag="z", bufs=16, name=f"z{h}")
                    nc.vector.tensor_mul(z, expT, eu_sb[:, ds(q0, NQ)])
                    st[ci].setdefault("z", {})[h] = z

                def emit_d(ci, h):
                    d_ps = ops.tile([1, NQ], F32, tag="dps", bufs=2, name=f"dps{h}")
                    nc.tensor.matmul(d_ps, ones_col, st[ci]["z"][h], start=True, stop=True)
                    return d_ps

                def sm_head2(ci, h, d_ps, d_next):
                    prob = prob_bufs[ci % 2]
                    z = st[ci]["z"][h]
                    dinv = soft.tile([1, NQ], F32, tag="dinv", bufs=2)
                    nc.vector.reciprocal_approx_fast(dinv, d_ps)
                    dinv_bf = soft.tile([1, NQ], BF16, tag="dinvbf", bufs=2)
                    nc.scalar.copy(dinv_bf, dinv)
                    nxt = emit_d(ci, h + 1) if d_next else None
                    db_ps = ops.tile([T, NQ], F32, tag="db", bufs=1)
                    nc.tensor.matmul(db_ps, ones_row, dinv_bf, start=True, stop=True)
                    p_stg = soft.tile([T, NQ], BF16, tag="pstg", bufs=4)
                    nc.vector.tensor_mul(p_stg, z, db_ps)
                    for (ti, pb, s0, nr) in _pack_pieces(h):
                        nc.sync.dma_start(
                            prob[ti][ds(pb, nr), :], p_stg[ds(s0, nr), :]
                        )
                    return nxt

                def av(ci):
                    q0 = NQ * ci
                    prob = prob_bufs[ci % 2]
                    hs_f = st[ci]["hs_f"]
                    for qj in range(NQ // 128):
                        for nh in range(2):
                            o_ps = ops.tile([128, 512], F32, tag="ops", bufs=2)
                            for kt in range(NKT):
                                nc.tensor.matmul(
                                    o_ps,
                                    prob[kt][:, ds(128 * qj, 128)],
                                    m_tiles[kt][:, ds(512 * nh, 512)],
                                    start=(kt == 0),
                                    stop=(kt == NKT - 1),
                                )
                            o_sb = work.tile([128, 512], F32, tag="osb", bufs=3)
                            nc.vector.tensor_add(
                                o_sb, o_ps, hs_f[:, qj, ds(512 * nh, 512)]
                            )
                            nc.sync.dma_start(
                                out[ds(q0 + 128 * qj, 128), ds(512 * nh, 512)],
                                o_sb,
                            )

                load(0)
                for ij in range(C // 128):
                    qt_group(0, ij)
                for ci in range(1, NCHUNK + 1):
                    if ci < NCHUNK:
                        load(ci)
                    for h in range(H):
                        sm_head1(ci - 1, h)
                        if ci < NCHUNK and h % 2 == 0:
                            qt_group(ci, h // 2)
                    d_cur = emit_d(ci - 1, 0)
                    for h in range(H):
                        d_cur = sm_head2(ci - 1, h, d_cur, h + 1 < H)
                    av(ci - 1)

    nc.compile()
    return nc


_NC_CACHE = {}


def get_nc():
    if "nc" not in _NC_CACHE:
        _NC_CACHE["nc"] = build_nc()
    return _NC_CACHE["nc"]


def _bf16(x):
    return np.asarray(x, dtype=ml_dtypes.bfloat16)


def make_in_maps(inputs):
    hs = np.ascontiguousarray(np.asarray(inputs["hidden_states"], dtype=np.float32))
    ehs = np.asarray(inputs["encoder_hidden_states"], dtype=np.float32)
    mask_A = np.asarray(inputs["mask_A"], dtype=np.float32)
    mask_B = np.asarray(inputs["mask_B"], dtype=np.float32)
    Wq = np.asarray(inputs["Wq"], dtype=np.float32)
    Wk = np.asarray(inputs["Wk"], dtype=np.float32)
    Wv = np.asarray(inputs["Wv"], dtype=np.float32)
    Wo = np.asarray(inputs["Wo"], dtype=np.float32)
    bo = np.asarray(inputs["bo"], dtype=np.float32)
    idxA = np.asarray(inputs["token_indices_A"]).astype(np.int64) % T
    idxB = np.asarray(inputs["token_indices_B"]).astype(np.int64) % T

    # suppression as a multiplicative mask: exp(col_bias)^T [77, HW].
    # col_bias "set" semantics: B overwrites A; rows not in A|B are exactly 1.
    col_bias = np.zeros((HW, T), np.float32)
    col_bias[:, idxA] = (-SUPPRESS * (1.0 - mask_A))[:, None]
    col_bias[:, idxB] = (-SUPPRESS * (1.0 - mask_B))[:, None]
    euabt = np.exp(col_bias.T)

    scale = 1.0 / np.sqrt(D)
    wq_bf = _bf16(Wq * scale)
    wk_bf, wv_bf, wo_bf = _bf16(Wk), _bf16(Wv), _bf16(Wo)
    euabt_bf = _bf16(euabt)
    bo_bf = _bf16(bo[None, :])

    in_maps = []
    for b in range(B):
        in_maps.append(
            {
                "hs": hs[b],
                "ehsT": _bf16(ehs[b].T.copy()),
                "wq": wq_bf,
                "wk": wk_bf,
                "wv": wv_bf,
                "wo": wo_bf,
                "euabt": euabt_bf,
                "bo": bo_bf,
            }
        )
    return in_maps


def kernel(**inputs) -> np.ndarray:
    from concourse.bass_utils import run_bass_kernel_spmd

    nc = get_nc()
    in_maps = make_in_maps(inputs)
    res = run_bass_kernel_spmd(nc, in_maps, core_ids=list(range(B)))
    return np.stack([res.results[b]["out"] for b in range(B)]).astype(np.float32)



# revision 13
# speedup vs baseline: 1.8641x; 1.8641x over previous
"""Trainium2 Bass kernel for nn_DenseAttnProcessor (sparse_attention), v2.

Cross-attention: q = hs@Wq, k/v = ehs@{Wk,Wv}, per-head softmax(qk^T/8 +
col_bias) @ v, @Wo + bo + residual.  B=8 batches -> data-parallel, one batch
per NeuronCore, no collectives.

v2 strategy (vs the bf16 v1 baseline at 711 us):
  * fp8e4 DoubleRow matmuls for the two 8.6-GFLOP GEMMs (q-projection and the
    stacked probs@[V@Wo] GEMM) and the k/v projections: halves PE streaming
    time.  Weights are host-scaled x64 into fp8's sweet spot; descales are
    folded into existing PSUM-evacuation copies (free).
  * probs are carried as 16*p in fp8 (p~1/77 would be subnormal), M rows as
    8*M, residual as 128*hs bf16 (host-prescaled); the kernel returns
    128*out and the host divides by 128 (exact power-of-2 scalings).
  * softmax normalization batched across heads: z (=exp(scores), fp8) is
    DMA-packed into the stacked [128,10,512] layout, the per-head denominators
    come from 5 DoubleRow selector matmuls -> Dhat [16,512] (sel value 1/16),
    ONE reciprocal + ONE bf16 copy per chunk (v1: 128 single-lane reciprocals
    and 128 tiny copies), then 10 selector-transpose broadcast matmuls + 10
    DVE multiplies produce the fp8 prob stack.  The suppression mask rides a
    host-precomputed fp8 exp(col_bias^T) stack multiplied in per k-tile.
  * hsT arrives host-pretransposed in fp8 (v1 burned scalar-engine casts and
    32 on-chip DMA transposes); residual arrives bf16.  HBM traffic drops
    from ~48MB to ~38MB per core.
  * software pipeline: at iter ci the PE stream interleaves scores(ci),
    qT(ci+1) DoubleRow groups, the lag-1 AV(ci-1) groups, and the batched
    normalization matmuls of ci, so the PE never starves and the HAM clock
    gate stays at 8/8 (v1 spent 348us at half clock).
"""

import sys

for _p in ("/opt/trn_rl_repo",):
    if _p not in sys.path:
        sys.path.insert(0, _p)

import numpy as np
import ml_dtypes

import concourse.mybir as mybir
import concourse.tile as tile
from concourse import bacc
from concourse.bass import ds
from concourse.masks import make_identity

F32 = mybir.dt.float32
BF16 = mybir.dt.bfloat16
F8 = mybir.dt.float8e4
AF = mybir.ActivationFunctionType

B, HW, C, CT, T, H, D = 8, 4096, 1024, 2048, 77, 16, 64
SUPPRESS = 20.0
RT = H * T + 1                # 1233 stacked rows (16*77 head rows + bo row)
NKT = (RT + 127) // 128       # 10 stack tiles
NQ = 512                      # q rows per chunk
NCHUNK = HW // NQ             # 8
BO_TILE, BO_PART = (H * T) // 128, (H * T) % 128   # bo/ones row: tile 9, part 80

NP_F8 = ml_dtypes.float8_e4m3
NP_BF = ml_dtypes.bfloat16
OUT_SCALE = 1.0 / 128.0  # device returns 128*(attn + bo + residual)


def _pack_pieces(h):
    """DMA pieces for packing head h's 77 rows at stacked row 77*h, split at
    128-row tile boundaries: list of (tile_idx, part_base, src_start, nrows)."""
    g = T * h
    pieces = []
    pos = 0
    while pos < T:
        gg = g + pos
        ti, d = gg // 128, gg % 128
        n = min(T - pos, 128 - d)
        pieces.append((ti, d, pos, n))
        pos += n
    return pieces


# stack tile kt is fully packed once head _KT_READY[kt] has been packed
_KT_READY = {}
for _kt in range(NKT):
    last_row = min(128 * _kt + 127, H * T - 1)
    _KT_READY.setdefault(last_row // T, []).append(_kt)


def build_nc():
    nc = bacc.Bacc("TRN2", target_bir_lowering=False, debug=False)

    hsT8 = nc.dram_tensor("hsT8", [128, C // 128, HW], F8, kind="ExternalInput")
    hsr = nc.dram_tensor("hsr", [128, HW // 128, C], BF16, kind="ExternalInput")
    wq8 = nc.dram_tensor("wq8", [128, C // 128, C], F8, kind="ExternalInput")
    wk8 = nc.dram_tensor("wk8", [128, CT // 128, C], F8, kind="ExternalInput")
    wv8 = nc.dram_tensor("wv8", [128, CT // 128, C], F8, kind="ExternalInput")
    wob = nc.dram_tensor("wob", [128, C // 128, C], BF16, kind="ExternalInput")
    # inner dim padded 77->80: DoubleRow ldweights requires pair-stride % 16 == 0
    ehsT8 = nc.dram_tensor("ehsT8", [128, CT // 128, 80], F8, kind="ExternalInput")
    eus8 = nc.dram_tensor("eus8", [128, NKT, HW], F8, kind="ExternalInput")
    sel8 = nc.dram_tensor("sel8", [128, NKT, H], F8, kind="ExternalInput")
    selT = nc.dram_tensor("selT", [16, NKT, 128], BF16, kind="ExternalInput")
    bo8 = nc.dram_tensor("bo8", [1, C], F8, kind="ExternalInput")
    ones16 = nc.dram_tensor("ones16", [1, NQ], F8, kind="ExternalInput")
    out = nc.dram_tensor("out", [HW, C], F32, kind="ExternalOutput")

    with tile.TileContext(nc) as tc:
        with (
            tc.tile_pool(name="const", bufs=1) as const,
            tc.tile_pool(name="persist", bufs=1) as persist,
        ):
            ident = const.tile([128, 128], BF16)
            make_identity(nc, ident)
            sel_sb = const.tile([128, NKT, H], F8)
            nc.sync.dma_start(sel_sb, sel8[:, :, :])
            selT_sb = const.tile([16, NKT, 128], BF16)
            nc.sync.dma_start(selT_sb, selT[:, :, :])

            kT_sb = persist.tile([128, C // 128, T], BF16)
            m8_sb = persist.tile([128, NKT, C], F8)
            wq_sb = persist.tile([128, C // 128, C], F8)
            nc.sync.dma_start(wq_sb, wq8[:, :, :])
            # stacked z / z*eu / prob buffers, parity double-buffered
            zstack = [persist.tile([128, NKT, NQ], F8, name=f"zst{b}") for b in range(2)]
            zs = [persist.tile([128, NKT, NQ], F8, name=f"zs{b}") for b in range(2)]
            prob = [persist.tile([128, NKT, NQ], F8, name=f"prob{b}") for b in range(2)]
            # garbage partitions beyond the packed rows must be zero: they meet
            # sel=0 / m=0 weights, and fp8 NaN garbage would poison 0*NaN.
            # (engine memsets must start at a x32 partition; rows 64:80 are
            # re-written by the packs / mul1 each chunk, so zeroing 64:128 is
            # fine.  The 16.0 ones-row itself comes in via DMA.)
            for b in range(2):
                nc.any.memset(zstack[b][ds(64, 64), BO_TILE, :], 0.0)
                nc.any.memset(zs[b][ds(64, 64), BO_TILE, :], 0.0)
                nc.any.memset(prob[b][ds(64, 64), BO_TILE, :], 0.0)
                # ones-row carries 16.0 (prob scale) so the bo row (8*bo) lands
                # as 128*bo in the psum, matching the 128x output scaling.
                nc.sync.dma_start(
                    prob[b][ds(BO_PART, 1), BO_TILE, :], ones16[:, :]
                )
            nc.any.memset(m8_sb[ds(64, 64), BO_TILE, :], 0.0)
            nc.sync.dma_start(m8_sb[ds(BO_PART, 1), BO_TILE, :], bo8[:, :])

            # ---------------- stage A: k, v, kT, vT, M ----------------
            with (
                tc.tile_pool(name="sa_sb", bufs=1) as sa_sb,
                tc.tile_pool(name="sa_ps", bufs=2, space="PSUM") as sa_ps,
            ):
                ehsT_sb = sa_sb.tile([128, CT // 128, 80], F8)
                nc.sync.dma_start(ehsT_sb, ehsT8[:, :, :])
                wk_sb = sa_sb.tile([128, CT // 128, C], F8)
                nc.sync.dma_start(wk_sb, wk8[:, :, :])
                wv_sb = sa_sb.tile([128, CT // 128, C], F8)
                nc.sync.dma_start(wv_sb, wv8[:, :, :])
                wo_sb = sa_sb.tile([128, C // 128, C], BF16)
                nc.sync.dma_start(wo_sb, wob[:, :, :])

                kv_sb = {}
                for name, wten in (("k", wk_sb), ("v", wv_sb)):
                    kv_ps = sa_ps.tile([T, C], F32, tag="kvps", bufs=1)
                    for nh in range(2):
                        for j in range(CT // 256):
                            nc.tensor.matmul(
                                kv_ps[:, ds(512 * nh, 512)],
                                ehsT_sb[:, ds(2 * j, 2), :T],
                                wten[:, ds(2 * j, 2), ds(512 * nh, 512)],
                                start=(j == 0),
                                stop=(j == CT // 256 - 1),
                                perf_mode=mybir.MatmulPerfMode.DoubleRow,
                            )
                    kvs = sa_sb.tile([T, C], BF16, tag=f"{name}sb", bufs=1)
                    # khat = 64*k -> bf16 k via 1/64 descale on evacuation
                    nc.scalar.activation(kvs, kv_ps, AF.Copy, scale=1.0 / 64.0)
                    kv_sb[name] = kvs

                vT_sb = sa_sb.tile([128, C // 128, T], BF16)
                for src, dst in ((kv_sb["k"], kT_sb), (kv_sb["v"], vT_sb)):
                    for i in range(C // 128):
                        tp = sa_ps.tile([128, T], BF16, tag="tpa", bufs=2)
                        nc.tensor.transpose(tp, src[:, ds(128 * i, 128)], ident[:T, :T])
                        nc.any.tensor_copy(dst[:, i, :], tp)

                # M_h = v_h @ (64*Wo_h); evacuate at 1/8 -> m8 = 8*M fp8
                for h in range(H):
                    i, po = h // 2, (h % 2) * 64
                    m_ps = sa_ps.tile([T, C], F32, tag="mps", bufs=2)
                    for nh in range(2):
                        nc.tensor.matmul(
                            m_ps[:, ds(512 * nh, 512)],
                            vT_sb[ds(po, 64), i, :],
                            wo_sb[ds(po, 64), i, ds(512 * nh, 512)],
                            start=True,
                            stop=True,
                        )
                    m_stg = sa_sb.tile([T, C], F8, tag="mstg", bufs=2)
                    nc.scalar.activation(m_stg, m_ps, AF.Copy, scale=1.0 / 8.0)
                    for (ti, pb, s0, nr) in _pack_pieces(h):
                        nc.sync.dma_start(
                            m8_sb[ds(pb, nr), ti, :], m_stg[ds(s0, nr), :]
                        )

            # ---------------- stage B: software-pipelined q chunks ----------------
            with (
                tc.tile_pool(name="ld", bufs=2) as ld,
                tc.tile_pool(name="work", bufs=2) as work,
                tc.tile_pool(name="soft", bufs=4) as soft,
                tc.tile_pool(name="spt", bufs=3, space="PSUM") as spt,
                tc.tile_pool(name="gemm", bufs=2, space="PSUM") as gemm,
                tc.tile_pool(name="dps", bufs=1, space="PSUM") as dps,
                tc.tile_pool(name="bcp", bufs=2, space="PSUM") as bcp,
            ):
                st = {}

                def load_hsT(ci):
                    hsT_t = ld.tile([128, C // 128, NQ], F8, tag="hsT")
                    nc.sync.dma_start(hsT_t, hsT8[:, :, ds(NQ * ci, NQ)])
                    st.setdefault(ci, {})["hsT"] = hsT_t

                def load_hsr_eu(ci):
                    hsr_t = ld.tile([128, NQ // 128, C], BF16, tag="hsr")
                    nc.sync.dma_start(hsr_t, hsr[:, ds(4 * ci, 4), :])
                    eu_t = ld.tile([128, NKT, NQ], F8, tag="eu")
                    nc.sync.dma_start(eu_t, eus8[:, :, ds(NQ * ci, NQ)])
                    st.setdefault(ci, {})
                    st[ci]["hsr"] = hsr_t
                    st[ci]["eu"] = eu_t

                def qt_group(ci, ij):
                    """qT rows [128*ij, 128*(ij+1)) for chunk ci: 4 DoubleRow MMs."""
                    d = st[ci]
                    if "qT" not in d:
                        d["qT"] = work.tile(
                            [128, C // 128, NQ], BF16, tag="qT", name=f"qT{ci}"
                        )
                    q_ps = gemm.tile([128, NQ], F32, tag="gps", name=f"qps{ci}_{ij}")
                    for j in range(C // 256):
                        nc.tensor.matmul(
                            q_ps,
                            wq_sb[:, ds(2 * j, 2), ds(128 * ij, 128)],
                            d["hsT"][:, ds(2 * j, 2), :],
                            start=(j == 0),
                            stop=(j == C // 256 - 1),
                            perf_mode=mybir.MatmulPerfMode.DoubleRow,
                        )
                    # qhatT = 512*qT -> bf16 qT/8 (descale + attn scale folded)
                    nc.vector.tensor_scalar_mul(d["qT"][:, ij, :], q_ps, 1.0 / 512.0)

                def sm_head(ci, h):
                    """scores + exp + pack for head h of chunk ci."""
                    q0 = NQ * ci
                    i, po = h // 2, (h % 2) * 64
                    sT_ps = spt.tile([T, NQ], F32, tag="sT", name=f"sT{ci}_{h}")
                    nc.tensor.matmul(
                        sT_ps,
                        kT_sb[ds(po, 64), i, :],
                        st[ci]["qT"][ds(po, 64), i, :],
                        start=True,
                        stop=True,
                    )
                    z8 = soft.tile([T, NQ], F8, tag="z8", bufs=4, name=f"z8_{h}")
                    nc.scalar.activation(z8, sT_ps, AF.Exp)
                    zst = zstack[ci % 2]
                    for (ti, pb, s0, nr) in _pack_pieces(h):
                        nc.sync.dma_start(zst[ds(pb, nr), ti, :], z8[ds(s0, nr), :])

                def mul1(ci, kt):
                    """zs = z * exp(col_bias) for stack tile kt (suppression)."""
                    par = ci % 2
                    rows = BO_PART if kt == BO_TILE else 128
                    nc.vector.tensor_mul(
                        zs[par][ds(0, rows), kt, :],
                        zstack[par][ds(0, rows), kt, :],
                        st[ci]["eu"][ds(0, rows), kt, :],
                    )

                def dhat_mm(ci, t, nt):
                    """Dhat [16, NQ] accumulation: pair t of the selector GEMM."""
                    d = st[ci]
                    if "dhat" not in d:
                        d["dhat"] = dps.tile([16, NQ], F32, tag="dh", name=f"dh{ci}")
                    nc.tensor.matmul(
                        d["dhat"],
                        sel_sb[:, ds(2 * t, 2), :],
                        zs[ci % 2][:, ds(2 * t, 2), :],
                        start=(t == 0),
                        stop=(t == nt - 1),
                        perf_mode=mybir.MatmulPerfMode.DoubleRow,
                    )

                def norm_head_scalars(ci):
                    """One reciprocal + one bf16 copy for all 16 heads."""
                    dinv = soft.tile([16, NQ], F32, tag="dinv", bufs=2)
                    nc.vector.reciprocal_approx_fast(dinv, st[ci]["dhat"])
                    dinv_bf = soft.tile([16, NQ], BF16, tag="dinvbf", bufs=2)
                    nc.scalar.activation(dinv_bf, dinv, AF.Copy)
                    st[ci]["dinv_bf"] = dinv_bf

                def bc_mul2(ci, kt):
                    """prob[kt] = zs[kt] * broadcast(dinv): selector-T matmul + mul."""
                    bc_ps = bcp.tile([128, NQ], F32, tag="bc", name=f"bc{ci}_{kt}")
                    nc.tensor.matmul(
                        bc_ps,
                        selT_sb[:, kt, :],
                        st[ci]["dinv_bf"],
                        start=True,
                        stop=True,
                    )
                    par = ci % 2
                    rows = BO_PART if kt == BO_TILE else 128
                    nc.vector.tensor_mul(
                        prob[par][ds(0, rows), kt, :],
                        zs[par][ds(0, rows), kt, :],
                        bc_ps[ds(0, rows), :],
                    )

                def av_group(ci, g):
                    """output block (qj, nh) = divmod(g, 2) of chunk ci."""
                    qj, nh = divmod(g, 2)
                    q0 = NQ * ci
                    pr = prob[ci % 2]
                    o_ps = gemm.tile([128, 512], F32, tag="gps", name=f"ops{ci}_{g}")
                    for t in range(NKT // 2):
                        nc.tensor.matmul(
                            o_ps,
                            pr[:, ds(2 * t, 2), ds(128 * qj, 128)],
                            m8_sb[:, ds(2 * t, 2), ds(512 * nh, 512)],
                            start=(t == 0),
                            stop=(t == NKT // 2 - 1),
                            perf_mode=mybir.MatmulPerfMode.DoubleRow,
                        )
                    o_sb = work.tile([128, 512], F32, tag="osb", bufs=3, name=f"osb{g}")
                    nc.vector.tensor_add(
                        o_sb, o_ps, st[ci]["hsr"][:, qj, ds(512 * nh, 512)]
                    )
                    nc.sync.dma_start(
                        out[ds(q0 + 128 * qj, 128), ds(512 * nh, 512)], o_sb
                    )

                # -------- prologue --------
                load_hsT(0)
                load_hsT(1)
                load_hsr_eu(0)
                for ij in range(C // 128):
                    qt_group(0, ij)

                # -------- steady-state iterations --------
                for ci in range(NCHUNK):
                    if ci + 2 < NCHUNK:
                        load_hsT(ci + 2)
                    if ci + 1 < NCHUNK:
                        load_hsr_eu(ci + 1)
                    for p in range(8):  # head pairs
                        for h in (2 * p, 2 * p + 1):
                            sm_head(ci, h)
                            for kt in _KT_READY.get(h, []):
                                mul1(ci, kt)
                                if kt % 2 == 1:
                                    dhat_mm(ci, kt // 2, NKT // 2)
                        if ci + 1 < NCHUNK:
                            qt_group(ci + 1, p)
                        if ci > 0 and p < 5:
                            av_group(ci - 1, p)
                    norm_head_scalars(ci)
                    for kt in range(NKT):
                        bc_mul2(ci, kt)
                        if ci > 0 and 5 + (kt // 4) < 8 and kt % 4 == 3:
                            av_group(ci - 1, 5 + kt // 4)
                    if ci > 0:
                        av_group(ci - 1, 7)
                    # drop chunk state no longer needed
                    if ci > 0:
                        st.pop(ci - 1, None)

                # -------- epilogue: AV of the last chunk --------
                for g in range(8):
                    av_group(NCHUNK - 1, g)

    nc.compile()
    return nc


_NC_CACHE = {}


def get_nc():
    if "nc" not in _NC_CACHE:
        _NC_CACHE["nc"] = build_nc()
    return _NC_CACHE["nc"]


def _f8(x):
    return np.clip(np.asarray(x, np.float32), -240.0, 240.0).astype(NP_F8)


def _bf(x):
    return np.asarray(x, dtype=NP_BF)


def _tile_rows(a, p=128):
    """[R, N] -> [p, R//p, N] with [r%p ... ] layout: out[q, j, n] = a[j*p+q, n]."""
    R, N = a.shape
    return np.ascontiguousarray(a.reshape(R // p, p, N).transpose(1, 0, 2))


def make_in_maps(inputs):
    hs = np.asarray(inputs["hidden_states"], dtype=np.float32)
    ehs = np.asarray(inputs["encoder_hidden_states"], dtype=np.float32)
    mask_A = np.asarray(inputs["mask_A"], dtype=np.float32)
    mask_B = np.asarray(inputs["mask_B"], dtype=np.float32)
    Wq = np.asarray(inputs["Wq"], dtype=np.float32)
    Wk = np.asarray(inputs["Wk"], dtype=np.float32)
    Wv = np.asarray(inputs["Wv"], dtype=np.float32)
    Wo = np.asarray(inputs["Wo"], dtype=np.float32)
    bo = np.asarray(inputs["bo"], dtype=np.float32)
    idxA = np.asarray(inputs["token_indices_A"]).astype(np.int64) % T
    idxB = np.asarray(inputs["token_indices_B"]).astype(np.int64) % T

    # suppression as multiplicative mask exp(col_bias)^T [T, HW]
    col_bias = np.zeros((HW, T), np.float32)
    col_bias[:, idxA] = (-SUPPRESS * (1.0 - mask_A))[:, None]
    col_bias[:, idxB] = (-SUPPRESS * (1.0 - mask_B))[:, None]
    eu = np.exp(col_bias.T)  # [T, HW]

    # stacked-layout tensors [128, NKT, *]
    eus = np.zeros((128, NKT, HW), np.float32)
    sel = np.zeros((128, NKT, H), np.float32)
    selTm = np.zeros((16, NKT, 128), np.float32)
    for r in range(H * T):
        kt, p = divmod(r, 128)
        h, t = divmod(r, T)[0], r % T
        eus[p, kt, :] = eu[t, :]
        sel[p, kt, h] = 1.0 / 16.0
        selTm[h, kt, p] = 1.0
    eus8_np = _f8(eus)
    sel8_np = _f8(sel)
    selT_np = _bf(selTm)

    wq8_np = _f8(_tile_rows(Wq * 64.0))
    wk8_np = _f8(_tile_rows(Wk * 64.0))
    wv8_np = _f8(_tile_rows(Wv * 64.0))
    wob_np = _bf(_tile_rows(Wo * 64.0))
    bo8_np = _f8(8.0 * bo)[None, :]

    in_maps = []
    for b in range(B):
        hsT = np.ascontiguousarray(hs[b].T)          # [C, HW]
        in_maps.append(
            {
                "hsT8": _f8(_tile_rows(hsT)),
                "hsr": _bf(_tile_rows(hs[b]) * 128.0),
                "wq8": wq8_np,
                "wk8": wk8_np,
                "wv8": wv8_np,
                "wob": wob_np,
                "ehsT8": np.pad(
                    _f8(_tile_rows(ehs[b].T.copy())), ((0, 0), (0, 0), (0, 3))
                ),
                "eus8": eus8_np,
                "sel8": sel8_np,
                "selT": selT_np,
                "bo8": bo8_np,
                "ones16": np.full((1, NQ), 16.0, NP_F8),
            }
        )
    return in_maps


def kernel(**inputs) -> np.ndarray:
    from concourse.bass_utils import run_bass_kernel_spmd

    nc = get_nc()
    in_maps = make_in_maps(inputs)
    res = run_bass_kernel_spmd(nc, in_maps, core_ids=list(range(B)))
    return (
        np.stack([res.results[b]["out"] for b in range(B)]).astype(np.float32)
        / 128.0
    )


# revision 20
# speedup vs baseline: 2.0610x; 1.1056x over previous
"""Trainium2 Bass kernel for nn_DenseAttnProcessor (sparse_attention), v3.

Cross-attention: q = hs@Wq, k/v = ehs@{Wk,Wv}, per-head softmax(qk^T/8 +
col_bias) @ v, @Wo + bo + residual.  B=8 batches -> data-parallel, one batch
per NeuronCore, no collectives.

Key design (see git history for the bf16 v1 at 711us / fp8 v2 at 382us):
  * fp8e4 DoubleRow matmuls for the two 8.6-GFLOP GEMMs (q-projection and the
    stacked probs@[V@Wo] GEMM) and the k/v projections.  Weights host-scaled
    x64 into fp8's sweet spot; descales folded into PSUM-evacuation copies.
  * probs carried as 16*p fp8 (p~1/77 would be fp8-subnormal), M rows as 8*M,
    residual as 128*hs bf16; kernel returns 128*out, host divides by 128.
  * batched softmax normalization: z=exp(scores) packs into the stacked
    [128,10,NQ] layout; per-head denominators via 5 DoubleRow selector
    matmuls -> Dhat [16,NQ]; ONE reciprocal + ONE bf16 copy per chunk; the
    inverse is broadcast back by 10 selector-transpose matmuls and applied by
    10 DVE multiplies.
  * the suppression mask exp(col_bias) is fused into the z pack itself: the
    stacked zs buffer is DMA-prefilled with the host-precomputed fp8 mask
    stack and the pack DMAs run on the gpsimd SWDGE with accum_op=mult
    (zs = eu * z), so no DVE instruction touches the mask at all.  The packs
    also live on the otherwise-idle gpsimd queue, off the sync engine.
  * engine balance: exp on scalar, qT/k/v/M-evacuations on scalar, reciprocal
    + prob-multiplies + residual adds on vector, packs on gpsimd, loads/stores
    on sync.  PE stream per iter interleaves scores(ci), qT(ci+1) DoubleRow
    groups, the lag-1 broadcast matmuls of ci-1, and the lag-1 AV groups so
    the PE never waits on the softmax tail and HAM stays at 8/8.
"""

import sys

for _p in ("/opt/trn_rl_repo",):
    if _p not in sys.path:
        sys.path.insert(0, _p)

import numpy as np
import ml_dtypes

import concourse.mybir as mybir
import concourse.tile as tile
from concourse import bacc
from concourse.bass import ds
from concourse.masks import make_identity

F32 = mybir.dt.float32
BF16 = mybir.dt.bfloat16
F8 = mybir.dt.float8e4
AF = mybir.ActivationFunctionType
DR = mybir.MatmulPerfMode.DoubleRow

B, HW, C, CT, T, H, D = 8, 4096, 1024, 2048, 77, 16, 64
SUPPRESS = 20.0
RT = H * T + 1                # 1233 stacked rows (16*77 head rows + bo row)
NKT = (RT + 127) // 128       # 10 stack tiles
NQ = 512                      # q rows per chunk
NCHUNK = HW // NQ             # 8
BO_TILE, BO_PART = (H * T) // 128, (H * T) % 128   # bo/ones row: tile 9, part 80

NP_F8 = ml_dtypes.float8_e4m3
NP_BF = ml_dtypes.bfloat16
OUT_SCALE = 1.0 / 128.0  # device returns 128*(attn + bo + residual)

# fuse the mask multiply into the pack DMA (SWDGE CCE mult) -- the CoreSim
# supports it but the hardware DMACopy rejects mult, so it stays off and the
# mask multiplies run per stack tile, alternating vector/gpsimd engines.
PACK_MULT = False


def _pack_pieces(h):
    """DMA pieces for packing head h's 77 rows at stacked row 77*h, split at
    128-row tile boundaries: list of (tile_idx, part_base, src_start, nrows)."""
    g = T * h
    pieces = []
    pos = 0
    while pos < T:
        gg = g + pos
        ti, d = gg // 128, gg % 128
        n = min(T - pos, 128 - d)
        pieces.append((ti, d, pos, n))
        pos += n
    return pieces


# stack tile kt is fully packed once head _KT_LAST_HEAD[kt] has been packed
_KT_LAST_HEAD = {kt: min(128 * kt + 127, H * T - 1) // T for kt in range(NKT)}
# Dhat pair t ready after head _KT_LAST_HEAD[2t+1]; emit its matmul two heads
# later so the ~1.5us SWDGE pack latency never stalls the PE stream.  Pairs
# whose slot would land past head 13 are emitted in the iter tail instead.
_DHAT_EMIT = {}
_DHAT_TAIL = []
for _t in range(NKT // 2):
    _eh = _KT_LAST_HEAD[2 * _t + 1] + 2
    if _eh <= 13:
        _DHAT_EMIT.setdefault(_eh, []).append(_t)
    else:
        _DHAT_TAIL.append(_t)
# broadcasts of the lag-1 chunk spread over the first four head pairs
_BC_PLAN = {0: 3, 1: 2, 2: 2, 3: 3}


def build_nc():
    nc = bacc.Bacc("TRN2", target_bir_lowering=False, debug=False)

    hsT8 = nc.dram_tensor("hsT8", [128, C // 128, HW], F8, kind="ExternalInput")
    hsr = nc.dram_tensor("hsr", [128, HW // 128, C], BF16, kind="ExternalInput")
    wq8 = nc.dram_tensor("wq8", [128, C // 128, C], F8, kind="ExternalInput")
    wk8 = nc.dram_tensor("wk8", [128, CT // 128, C], F8, kind="ExternalInput")
    wv8 = nc.dram_tensor("wv8", [128, CT // 128, C], F8, kind="ExternalInput")
    wob = nc.dram_tensor("wob", [128, C // 128, C], BF16, kind="ExternalInput")
    # inner dim padded 77->80: DoubleRow ldweights requires pair-stride % 16 == 0
    ehsT8 = nc.dram_tensor("ehsT8", [128, CT // 128, 80], F8, kind="ExternalInput")
    eus8 = nc.dram_tensor("eus8", [128, NKT, HW], F8, kind="ExternalInput")
    sel8 = nc.dram_tensor("sel8", [128, NKT, H], F8, kind="ExternalInput")
    selT = nc.dram_tensor("selT", [16, NKT, 128], BF16, kind="ExternalInput")
    bo8 = nc.dram_tensor("bo8", [1, C], F8, kind="ExternalInput")
    ones16 = nc.dram_tensor("ones16", [1, NQ], F8, kind="ExternalInput")
    out = nc.dram_tensor("out", [HW, C], F32, kind="ExternalOutput")

    with tile.TileContext(nc) as tc:
        with (
            tc.tile_pool(name="const", bufs=1) as const,
            tc.tile_pool(name="persist", bufs=1) as persist,
        ):
            ident = const.tile([128, 128], BF16)
            make_identity(nc, ident)
            sel_sb = const.tile([128, NKT, H], F8)
            selT_sb = const.tile([16, NKT, 128], BF16)

            kT_sb = persist.tile([128, C // 128, T], BF16)
            m8_sb = persist.tile([128, NKT, C], F8)
            wq_sb = persist.tile([128, C // 128, C], F8)
            # stacked z*eu / prob buffers, parity double-buffered
            zs = [persist.tile([128, NKT, NQ], F8, name=f"zs{b}") for b in range(2)]
            prob = [persist.tile([128, NKT, NQ], F8, name=f"prob{b}") for b in range(2)]
            # garbage partitions beyond the packed rows must be zero: they meet
            # sel=0 / m=0 weights, and fp8 NaN garbage would poison 0*NaN.
            # (zs is fully covered by the eu prefill each chunk; for prob the
            # 16.0 ones-row comes in via DMA and rows 81:128 stay zero.)
            for b in range(2):
                nc.any.memset(prob[b][ds(64, 64), BO_TILE, :], 0.0)
                if not PACK_MULT:
                    nc.any.memset(zs[b][ds(64, 64), BO_TILE, :], 0.0)
                nc.sync.dma_start(
                    prob[b][ds(BO_PART, 1), BO_TILE, :], ones16[:, :]
                )
            nc.any.memset(m8_sb[ds(64, 64), BO_TILE, :], 0.0)
            nc.sync.dma_start(m8_sb[ds(BO_PART, 1), BO_TILE, :], bo8[:, :])

            # ---------------- stage A: k, v, kT, vT, M ----------------
            with (
                tc.tile_pool(name="sa_sb", bufs=1) as sa_sb,
                tc.tile_pool(name="sa_ps", bufs=2, space="PSUM") as sa_ps,
            ):
                # DMA emission order = sync-queue order: the two tensors the
                # first matmuls need come first, the rest overlaps compute.
                ehsT_sb = sa_sb.tile([128, CT // 128, 80], F8)
                nc.sync.dma_start(ehsT_sb, ehsT8[:, :, :])
                wk_sb = sa_sb.tile([128, CT // 128, C], F8)
                nc.sync.dma_start(wk_sb, wk8[:, :, :])
                wv_sb = sa_sb.tile([128, CT // 128, C], F8)
                nc.sync.dma_start(wv_sb, wv8[:, :, :])
                nc.sync.dma_start(wq_sb, wq8[:, :, :])
                wo_sb = sa_sb.tile([128, C // 128, C], BF16)
                nc.sync.dma_start(wo_sb, wob[:, :, :])
                nc.sync.dma_start(sel_sb, sel8[:, :, :])
                nc.sync.dma_start(selT_sb, selT[:, :, :])

                kv_sb = {}
                for name, wten in (("k", wk_sb), ("v", wv_sb)):
                    kv_ps = sa_ps.tile([T, C], F32, tag="kvps", bufs=1)
                    for nh in range(2):
                        for j in range(CT // 256):
                            nc.tensor.matmul(
                                kv_ps[:, ds(512 * nh, 512)],
                                ehsT_sb[:, ds(2 * j, 2), :T],
                                wten[:, ds(2 * j, 2), ds(512 * nh, 512)],
                                start=(j == 0),
                                stop=(j == CT // 256 - 1),
                                perf_mode=DR,
                            )
                    kvs = sa_sb.tile([T, C], BF16, tag=f"{name}sb", bufs=1)
                    # khat = 64*k -> bf16 k via 1/64 descale on evacuation
                    nc.scalar.activation(kvs, kv_ps, AF.Copy, scale=1.0 / 64.0)
                    kv_sb[name] = kvs

                vT_sb = sa_sb.tile([128, C // 128, T], BF16)
                for src, dst in ((kv_sb["k"], kT_sb), (kv_sb["v"], vT_sb)):
                    for i in range(C // 128):
                        tp = sa_ps.tile([128, T], BF16, tag="tpa", bufs=2)
                        nc.tensor.transpose(tp, src[:, ds(128 * i, 128)], ident[:T, :T])
                        nc.any.tensor_copy(dst[:, i, :], tp)

                # M_h = v_h @ (64*Wo_h); evacuate at 1/8 -> m8 = 8*M fp8
                for h in range(H):
                    i, po = h // 2, (h % 2) * 64
                    m_ps = sa_ps.tile([T, C], F32, tag="mps", bufs=2)
                    for nh in range(2):
                        nc.tensor.matmul(
                            m_ps[:, ds(512 * nh, 512)],
                            vT_sb[ds(po, 64), i, :],
                            wo_sb[ds(po, 64), i, ds(512 * nh, 512)],
                            start=True,
                            stop=True,
                        )
                    m_stg = sa_sb.tile([T, C], F8, tag="mstg", bufs=2)
                    nc.scalar.activation(m_stg, m_ps, AF.Copy, scale=1.0 / 8.0)
                    for (ti, pb, s0, nr) in _pack_pieces(h):
                        nc.gpsimd.dma_start(
                            m8_sb[ds(pb, nr), ti, :], m_stg[ds(s0, nr), :]
                        )

            # ---------------- stage B: software-pipelined q chunks ----------------
            with (
                tc.tile_pool(name="ld", bufs=2) as ld,
                tc.tile_pool(name="work", bufs=2) as work,
                tc.tile_pool(name="soft", bufs=4) as soft,
                tc.tile_pool(name="spt", bufs=3, space="PSUM") as spt,
                tc.tile_pool(name="gemm", bufs=2, space="PSUM") as gemm,
                tc.tile_pool(name="dps", bufs=1, space="PSUM") as dps,
                tc.tile_pool(name="bcp", bufs=2, space="PSUM") as bcp,
            ):
                st = {}

                def load_hsT(ci):
                    hsT_t = ld.tile([128, C // 128, NQ], F8, tag="hsT", name=f"ht{ci}")
                    nc.sync.dma_start(hsT_t, hsT8[:, :, ds(NQ * ci, NQ)])
                    st.setdefault(ci, {})["hsT"] = hsT_t

                def load_hsr(ci):
                    hsr_t = ld.tile([128, NQ // 128, C], BF16, tag="hsr", name=f"hr{ci}")
                    nc.sync.dma_start(hsr_t, hsr[:, ds(4 * ci, 4), :])
                    st.setdefault(ci, {})["hsr"] = hsr_t

                def prefill_eu(ci):
                    """zs := eu slice; the packs then multiply z in (CCE mult)."""
                    nc.sync.dma_start(
                        zs[ci % 2][:, :, :], eus8[:, :, ds(NQ * ci, NQ)]
                    )

                def load_eu(ci):
                    eu_t = ld.tile([128, NKT, NQ], F8, tag="eu", name=f"eu{ci}")
                    nc.sync.dma_start(eu_t, eus8[:, :, ds(NQ * ci, NQ)])
                    st.setdefault(ci, {})["eu"] = eu_t

                def qt_group(ci, ij):
                    """qT rows [128*ij, 128*(ij+1)) for chunk ci: 4 DoubleRow MMs."""
                    d = st[ci]
                    if "qT" not in d:
                        d["qT"] = work.tile(
                            [128, C // 128, NQ], BF16, tag="qT", name=f"qT{ci}"
                        )
                    q_ps = gemm.tile([128, NQ], F32, tag="gps", name=f"qps{ci}_{ij}")
                    for j in range(C // 256):
                        nc.tensor.matmul(
                            q_ps,
                            wq_sb[:, ds(2 * j, 2), ds(128 * ij, 128)],
                            d["hsT"][:, ds(2 * j, 2), :],
                            start=(j == 0),
                            stop=(j == C // 256 - 1),
                            perf_mode=DR,
                        )
                    # qhatT = 512*qT -> bf16 qT/8 (descale + attn scale folded)
                    nc.scalar.activation(
                        d["qT"][:, ij, :], q_ps, AF.Copy, scale=1.0 / 512.0
                    )

                def sm_head(ci, h):
                    """scores + exp + mask-fused pack for head h of chunk ci."""
                    i, po = h // 2, (h % 2) * 64
                    sT_ps = spt.tile([T, NQ], F32, tag="sT", name=f"sT{ci}_{h}")
                    nc.tensor.matmul(
                        sT_ps,
                        kT_sb[ds(po, 64), i, :],
                        st[ci]["qT"][ds(po, 64), i, :],
                        start=True,
                        stop=True,
                    )
                    z8 = soft.tile([T, NQ], F8, tag="z8", bufs=4, name=f"z8_{h}")
                    nc.scalar.activation(z8, sT_ps, AF.Exp)
                    zst = zs[ci % 2]
                    for (ti, pb, s0, nr) in _pack_pieces(h):
                        if PACK_MULT:
                            nc.gpsimd.dma_start(
                                zst[ds(pb, nr), ti, :],
                                z8[ds(s0, nr), :],
                                accum_op=mybir.AluOpType.mult,
                            )
                        else:
                            nc.gpsimd.dma_start(
                                zst[ds(pb, nr), ti, :], z8[ds(s0, nr), :]
                            )

                def mul1(ci, kt):
                    """fallback when PACK_MULT is off: zs *= eu per stack tile
                    (alternating engines so neither vector nor gpsimd paces)."""
                    par = ci % 2
                    rows = BO_PART if kt == BO_TILE else 128
                    eng = nc.vector if kt % 2 == 0 else nc.gpsimd
                    eng.tensor_mul(
                        zs[par][ds(0, rows), kt, :],
                        zs[par][ds(0, rows), kt, :],
                        st[ci]["eu"][ds(0, rows), kt, :],
                    )

                def dhat_mm(ci, t):
                    """Dhat [16, NQ] accumulation: pair t of the selector GEMM."""
                    d = st[ci]
                    if "dhat" not in d:
                        d["dhat"] = dps.tile([16, NQ], F32, tag="dh", name=f"dh{ci}")
                    nc.tensor.matmul(
                        d["dhat"],
                        sel_sb[:, ds(2 * t, 2), :],
                        zs[ci % 2][:, ds(2 * t, 2), :],
                        start=(t == 0),
                        stop=(t == NKT // 2 - 1),
                        perf_mode=DR,
                    )

                def norm_head_scalars(ci):
                    """One reciprocal + one bf16 copy for all 16 heads."""
                    dinv = soft.tile([16, NQ], F32, tag="dinv", bufs=2)
                    nc.vector.reciprocal_approx_fast(dinv, st[ci]["dhat"])
                    dinv_bf = soft.tile([16, NQ], BF16, tag="dinvbf", bufs=2)
                    nc.scalar.activation(dinv_bf, dinv, AF.Copy)
                    st[ci]["dinv_bf"] = dinv_bf

                def bc_mul2(ci, kt):
                    """prob[kt] = zs[kt] * broadcast(dinv): selector-T matmul + mul."""
                    bc_ps = bcp.tile([128, NQ], F32, tag="bc", name=f"bc{ci}_{kt}")
                    nc.tensor.matmul(
                        bc_ps,
                        selT_sb[:, kt, :],
                        st[ci]["dinv_bf"],
                        start=True,
                        stop=True,
                    )
                    par = ci % 2
                    rows = BO_PART if kt == BO_TILE else 128
                    nc.vector.tensor_mul(
                        prob[par][ds(0, rows), kt, :],
                        zs[par][ds(0, rows), kt, :],
                        bc_ps[ds(0, rows), :],
                    )

                def av_group(ci, g):
                    """output block (qj, nh) = divmod(g, 2) of chunk ci."""
                    qj, nh = divmod(g, 2)
                    q0 = NQ * ci
                    pr = prob[ci % 2]
                    o_ps = gemm.tile([128, 512], F32, tag="gps", name=f"ops{ci}_{g}")
                    for t in range(NKT // 2):
                        nc.tensor.matmul(
                            o_ps,
                            pr[:, ds(2 * t, 2), ds(128 * qj, 128)],
                            m8_sb[:, ds(2 * t, 2), ds(512 * nh, 512)],
                            start=(t == 0),
                            stop=(t == NKT // 2 - 1),
                            perf_mode=DR,
                        )
                    o_sb = work.tile([128, 512], F32, tag="osb", bufs=3, name=f"osb{g}")
                    nc.vector.tensor_add(
                        o_sb, o_ps, st[ci]["hsr"][:, qj, ds(512 * nh, 512)]
                    )
                    nc.sync.dma_start(
                        out[ds(q0 + 128 * qj, 128), ds(512 * nh, 512)], o_sb
                    )

                # -------- prologue --------
                load_hsT(0)
                load_hsT(1)
                load_hsr(0)
                if PACK_MULT:
                    prefill_eu(0)
                else:
                    load_eu(0)
                for ij in range(C // 128):
                    qt_group(0, ij)

                # -------- steady-state iterations --------
                # at iter ci the PE stream carries: scores(ci) pairs,
                # qT(ci+1) groups, bc(ci-1) + AV(ci-1) (both lag-1, fully
                # ready at iter start), and the Dhat(ci) chain (emitted two
                # heads behind the packs that feed it).
                for ci in range(NCHUNK):
                    if ci + 2 < NCHUNK:
                        load_hsT(ci + 2)
                    if ci + 1 < NCHUNK:
                        load_hsr(ci + 1)
                    bc_left = list(range(NKT))
                    for p in range(8):  # head pairs
                        for h in (2 * p, 2 * p + 1):
                            sm_head(ci, h)
                            if not PACK_MULT:
                                for kt in range(NKT):
                                    if _KT_LAST_HEAD[kt] == h:
                                        mul1(ci, kt)
                            for t in _DHAT_EMIT.get(h, []):
                                dhat_mm(ci, t)
                        if ci + 1 < NCHUNK:
                            qt_group(ci + 1, p)
                        if ci > 0:
                            if p < 4:  # 10 broadcasts over the first 4 pairs
                                for _ in range(_BC_PLAN[p]):
                                    bc_mul2(ci - 1, bc_left.pop(0))
                            else:  # AV groups 0-3 on pairs 4-7
                                av_group(ci - 1, p - 4)
                    if ci > 0:
                        for g in range(4, 8):
                            av_group(ci - 1, g)
                    for t in _DHAT_TAIL:
                        dhat_mm(ci, t)
                    norm_head_scalars(ci)
                    if ci + 1 < NCHUNK:
                        if PACK_MULT:
                            prefill_eu(ci + 1)
                        else:
                            load_eu(ci + 1)
                    if ci > 1:
                        st.pop(ci - 2, None)

                # -------- epilogue: norm + AV of the last chunk --------
                # (every AV matmul reads ALL stack tiles, so all broadcasts
                # must be emitted before the first AV group)
                ci = NCHUNK - 1
                for kt in range(NKT):
                    bc_mul2(ci, kt)
                for g in range(8):
                    av_group(ci, g)

    nc.compile()
    return nc


_NC_CACHE = {}


def get_nc():
    if "nc" not in _NC_CACHE:
        _NC_CACHE["nc"] = build_nc()
    return _NC_CACHE["nc"]


def _f8(x):
    return np.clip(np.asarray(x, np.float32), -240.0, 240.0).astype(NP_F8)


def _bf(x):
    return np.asarray(x, dtype=NP_BF)


def _tile_rows(a, p=128):
    """[R, N] -> [p, R//p, N] with out[q, j, n] = a[j*p+q, n]."""
    R, N = a.shape
    return np.ascontiguousarray(a.reshape(R // p, p, N).transpose(1, 0, 2))


def make_in_maps(inputs):
    hs = np.asarray(inputs["hidden_states"], dtype=np.float32)
    ehs = np.asarray(inputs["encoder_hidden_states"], dtype=np.float32)
    mask_A = np.asarray(inputs["mask_A"], dtype=np.float32)
    mask_B = np.asarray(inputs["mask_B"], dtype=np.float32)
    Wq = np.asarray(inputs["Wq"], dtype=np.float32)
    Wk = np.asarray(inputs["Wk"], dtype=np.float32)
    Wv = np.asarray(inputs["Wv"], dtype=np.float32)
    Wo = np.asarray(inputs["Wo"], dtype=np.float32)
    bo = np.asarray(inputs["bo"], dtype=np.float32)
    idxA = np.asarray(inputs["token_indices_A"]).astype(np.int64) % T
    idxB = np.asarray(inputs["token_indices_B"]).astype(np.int64) % T

    # suppression as multiplicative mask exp(col_bias)^T [T, HW]
    col_bias = np.zeros((HW, T), np.float32)
    col_bias[:, idxA] = (-SUPPRESS * (1.0 - mask_A))[:, None]
    col_bias[:, idxB] = (-SUPPRESS * (1.0 - mask_B))[:, None]
    eu = np.exp(col_bias.T)  # [T, HW]

    # stacked-layout tensors [128, NKT, *]
    eus = np.zeros((128, NKT, HW), np.float32)
    sel = np.zeros((128, NKT, H), np.float32)
    selTm = np.zeros((16, NKT, 128), np.float32)
    for r in range(H * T):
        kt, p = divmod(r, 128)
        h, t = r // T, r % T
        eus[p, kt, :] = eu[t, :]
        sel[p, kt, h] = 1.0 / 16.0
        selTm[h, kt, p] = 1.0
    eus8_np = _f8(eus)
    sel8_np = _f8(sel)
    selT_np = _bf(selTm)

    wq8_np = _f8(_tile_rows(Wq * 64.0))
    wk8_np = _f8(_tile_rows(Wk * 64.0))
    wv8_np = _f8(_tile_rows(Wv * 64.0))
    wob_np = _bf(_tile_rows(Wo * 64.0))
    bo8_np = _f8(8.0 * bo)[None, :]

    in_maps = []
    for b in range(B):
        hsT = np.ascontiguousarray(hs[b].T)          # [C, HW]
        in_maps.append(
            {
                "hsT8": _f8(_tile_rows(hsT)),
                "hsr": _bf(_tile_rows(hs[b]) * 128.0),
                "wq8": wq8_np,
                "wk8": wk8_np,
                "wv8": wv8_np,
                "wob": wob_np,
                "ehsT8": np.pad(
                    _f8(_tile_rows(ehs[b].T.copy())), ((0, 0), (0, 0), (0, 3))
                ),
                "eus8": eus8_np,
                "sel8": sel8_np,
                "selT": selT_np,
                "bo8": bo8_np,
                "ones16": np.full((1, NQ), 16.0, NP_F8),
            }
        )
    return in_maps


def kernel(**inputs) -> np.ndarray:
    from concourse.bass_utils import run_bass_kernel_spmd

    nc = get_nc()
    in_maps = make_in_maps(inputs)
    res = run_bass_kernel_spmd(nc, in_maps, core_ids=list(range(B)))
    return (
        np.stack([res.results[b]["out"] for b in range(B)]).astype(np.float32)
        * OUT_SCALE
    )
